# revision 9
# baseline (speedup 1.0000x reference)
"""GRU-decoder kernel for 8 Trainium2 NeuronCores (v3).

Math (all 127 output steps are identical -- see the reference):
    x0   = relu(emb[input[:,0]])                       [B,H]
    h0   = einsum('blh,l->bh', hidden, bridge_w) + bb  [B,H]
    gi   = x0 @ w_ih.T + b_ih ; gh = h0 @ w_hh.T + b_hh
    r,z  = sigmoid(...) ; n = tanh(in + r*hn)
    h1   = (1-z)*n + z*h0
    logp = log_softmax(h1 @ proj_w.T + proj_b)         [B,V]
    out  = broadcast(logp, [B, L-1, V])

Sharding (v3):
  - bridge: contraction over L; each core owns a 128-wide h-slice of
    `hidden`, produces h0T_own [128,16] directly in T layout.
  - AllGather #1 (4KB): h0 slices -> full h0T on every core.
  - gates: output-sharded (384 rows/core); weights are the moving
    operand, biases folded in as K=1 matmul rows.
  - AllGather #2 (4KB): h1 slices -> full h1T on every core.
  - projection: vocab-sharded; proj_w in fp8(e4m3, x2048) with
    DoubleRow matmuls (2 K-chunks per pass, 0.5 cyc/row); h1 cast to
    fp8 (x16); proj_b via a bf16 K=1 row (x32768). The 2^-15 descale
    is folded into the Exp activation scale on device and applied on
    host for the logits themselves. Host combines per-core exp-sums
    into the global log-softmax normalizer.
"""

import numpy as np
import ml_dtypes

import concourse.bass as bass
import concourse.tile as tile
from concourse import bacc, mybir
from concourse.bass_utils import run_bass_kernel_spmd

B, L, H, V = 16, 128, 1024, 50257
NC = 8
VC = 6656                # per-core vocab shard; 8*VC = 53248 >= V
KC = 8                   # contraction chunks of 128 over H
KK = 4                   # DoubleRow pairs of K-chunks
G3 = 384                 # per-core gate rows (3 x 128)
NEG = -1.0e30

PW_S = 2048.0            # proj_w fp8 scale
H1_S = 16.0              # h1 fp8 scale
LG_S = PW_S * H1_S       # logits scale (2^15)

f32 = mybir.dt.float32
bf16 = mybir.dt.bfloat16
f8 = mybir.dt.float8e4
FX = mybir.ActivationFunctionType
AX = mybir.AxisListType
DR = mybir.MatmulPerfMode.DoubleRow

BF = ml_dtypes.bfloat16
F8 = ml_dtypes.float8_e4m3

# projection groups: col ranges, chunks of 512
GROUPS = [(0, 2048), (2048, 4096), (4096, 6144), (6144, 6656)]

LAST_RESULT = None  # test harness reads profiling info from here
_NC_CACHE = None


def _bc(ap, insert_at, step, count):
    """Insert a broadcast/strided dim into an AP at position insert_at."""
    new = list(ap.ap)
    new.insert(insert_at, [step, count])
    return bass.AP(tensor=ap.tensor, offset=ap.offset, ap=new)


def _build():
    nc = bacc.Bacc("TRN2", target_bir_lowering=False, debug=False, num_devices=NC)

    hid = nc.dram_tensor("hid", [L, B, 128], bf16, kind="ExternalInput").ap()
    bw = nc.dram_tensor("bw", [L, 1], bf16, kind="ExternalInput").ap()
    x0T = nc.dram_tensor("x0T", [128, KC, B], bf16, kind="ExternalInput").ap()
    wih = nc.dram_tensor("wih", [128, KC, G3], bf16, kind="ExternalInput").ap()
    whh = nc.dram_tensor("whh", [128, KC, G3], bf16, kind="ExternalInput").ap()
    brow = nc.dram_tensor("brow", [1, 512], f32, kind="ExternalInput").ap()
    bb = nc.dram_tensor("bb", [1, 1], f32, kind="ExternalInput").ap()
    ones1 = nc.dram_tensor("ones1", [1, B], bf16, kind="ExternalInput").ap()
    pwT = nc.dram_tensor("pwT", [KK, 128, 2, VC], f8, kind="ExternalInput").ap()
    pb = nc.dram_tensor("pb", [1, VC], bf16, kind="ExternalInput").ap()
    idbf = nc.dram_tensor("idbf", [128, 128], bf16, kind="ExternalInput").ap()
    id16s = nc.dram_tensor("id16s", [B, B], f32, kind="ExternalInput").ap()
    logits = nc.dram_tensor("logits", [B, VC], bf16, kind="ExternalOutput").ap()
    svec = nc.dram_tensor("svec", [B, 1], f32, kind="ExternalOutput").ap()

    with tile.TileContext(nc) as tc:
        with (
            tc.tile_pool(name="singles", bufs=1) as singles,
            tc.tile_pool(name="dram", bufs=1, space="DRAM") as dram,
        ):
            # ---- bridge inputs first, then the pw stream (same HWDGE
            # ring -> FIFO: hid lands at full bandwidth before pw hogs it)
            hid_sb = singles.tile([L, B, 128], bf16, tag="hid_sb")
            nc.sync.dma_start(out=hid_sb, in_=hid)
            bw_sb = singles.tile([L, 1], bf16, tag="bw_sb")
            nc.sync.dma_start(out=bw_sb, in_=bw)
            bb_sb = singles.tile([128, 1], f32, tag="bb_sb")
            nc.sync.dma_start(out=bb_sb, in_=_bc(bb[0], 0, 0, 128))
            pw_sb = []
            for k in range(KK):
                t = singles.tile([128, 2, VC], f8, tag=f"pw{k}", name=f"pw{k}")
                nc.sync.dma_start(out=t, in_=pwT[k])
                pw_sb.append(t)
            x0T_sb = singles.tile([128, KC, B], bf16, tag="x0T_sb")
            nc.sync.dma_start(out=x0T_sb, in_=x0T)
            wih_sb = singles.tile([128, KC, G3], bf16, tag="wih_sb")
            nc.sync.dma_start(out=wih_sb, in_=wih)
            whh_sb = singles.tile([128, KC, G3], bf16, tag="whh_sb")
            nc.sync.dma_start(out=whh_sb, in_=whh)
            brow_sb = singles.tile([1, 512], f32, tag="brow_sb")
            nc.sync.dma_start(out=brow_sb, in_=brow)
            ones_sb = singles.tile([1, B], bf16, tag="ones_sb")
            nc.sync.dma_start(out=ones_sb, in_=ones1)
            onesf_sb = singles.tile([1, B], f32, tag="onesf_sb")
            nc.vector.memset(onesf_sb, 1.0)
            pb_sb = singles.tile([1, VC], bf16, tag="pb_sb")
            nc.sync.dma_start(out=pb_sb, in_=pb)

            # identities for PE transposes (host inputs; id16s = 16*I so
            # the h1 transpose-matmul also applies the fp8 scale)
            id128 = singles.tile([128, 128], bf16, tag="id128")
            nc.sync.dma_start(out=id128, in_=idbf)
            id16 = singles.tile([B, B], f32, tag="id16")
            nc.sync.dma_start(out=id16, in_=id16s)

            logits_sb = singles.tile([B, VC], bf16, tag="logits_sb")
            scratch = singles.tile([B, 2048], bf16, tag="scratch")
            cs_t = singles.tile([B, len(GROUPS)], f32, tag="cs_t")
            s_run = singles.tile([B, 1], f32, tag="s_run")

            h0T_own = singles.tile([128, B], bf16, tag="h0T_own")
            h0B_own = singles.tile([B, 128], f32, tag="h0B_own")
            h0T_full = singles.tile([128, KC, B], bf16, tag="h0T_full")
            h1f8 = singles.tile([128, KC, B], f8, tag="h1f8")
            trz = singles.tile([B, 256], f32, tag="trz")
            tn = singles.tile([B, 128], f32, tag="tn")
            td = singles.tile([B, 128], f32, tag="td")

            with tc.tile_pool(name="gru_ps", bufs=1, space="PSUM") as gps:
                # ---- bridge: h0T_own[h,b] = sum_l hid[l,b,h]*bw[l] -------
                h0T_ps = gps.tile([128, B], f32, tag="h0T_ps")
                for b in range(B):
                    nc.tensor.matmul(
                        h0T_ps[:, b : b + 1], hid_sb[:, b, :], bw_sb[:],
                        start=True, stop=True,
                    )
                nc.vector.tensor_scalar_add(h0T_own[:], h0T_ps[:], bb_sb[:, 0:1])

                # ---- AllGather #1: h0 slices -> full h0T -----------------
                cc1_in = dram.tile([128, B], bf16, tag="cc1_in")
                cc1_out = dram.tile([KC * 128, B], bf16, tag="cc1_out")
                nc.scalar.dma_start(out=cc1_in[:], in_=h0T_own[:])
                nc.gpsimd.collective_compute(
                    "AllGather",
                    mybir.AluOpType.bypass,
                    replica_groups=[list(range(NC))],
                    ins=[cc1_in.opt()],
                    outs=[cc1_out.opt()],
                )
                co = cc1_out[:]
                nc.scalar.dma_start(
                    out=h0T_full,
                    in_=bass.AP(
                        tensor=co.tensor, offset=co.offset,
                        ap=[[B, 128], [128 * B, KC], [1, B]],
                    ),
                )

                # own h0 slice in B layout for the h1 update
                h0B_ps = gps.tile([B, 128], bf16, tag="h0B_ps")
                nc.tensor.transpose(h0B_ps[:], h0T_own[:], id128[:])
                nc.vector.tensor_copy(h0B_own[:], h0B_ps[:])

                # ---- gates (output-sharded, B layout [16, 384]) ----------
                # psum accumulates gi + gh (+ bias row) per gate block
                grz_ps = gps.tile([B, 256], f32, tag="grz_ps")
                gin_ps = gps.tile([B, 128], f32, tag="gin_ps")
                ghn_ps = gps.tile([B, 128], f32, tag="ghn_ps")
                for k in range(KC):
                    nc.tensor.matmul(
                        grz_ps[:], x0T_sb[:, k, :], wih_sb[:, k, 0:256],
                        start=(k == 0), stop=False,
                    )
                    nc.tensor.matmul(
                        gin_ps[:], x0T_sb[:, k, :], wih_sb[:, k, 256:384],
                        start=(k == 0), stop=False,
                    )
                nc.tensor.matmul(
                    gin_ps[:], onesf_sb[:], brow_sb[0:1, 256:384],
                    start=False, stop=True,
                )
                for k in range(KC):
                    nc.tensor.matmul(
                        grz_ps[:], h0T_full[:, k, :], whh_sb[:, k, 0:256],
                        start=False, stop=False,
                    )
                    nc.tensor.matmul(
                        ghn_ps[:], h0T_full[:, k, :], whh_sb[:, k, 256:384],
                        start=(k == 0), stop=False,
                    )
                nc.tensor.matmul(
                    grz_ps[:], onesf_sb[:], brow_sb[0:1, 0:256],
                    start=False, stop=True,
                )
                nc.tensor.matmul(
                    ghn_ps[:], onesf_sb[:], brow_sb[0:1, 384:512],
                    start=False, stop=True,
                )

                # r,z = sigmoid(grz) ; n = tanh(gin + r*ghn)
                nc.scalar.activation(out=trz[:], in_=grz_ps[:], func=FX.Sigmoid)
                nc.vector.tensor_mul(tn[:], ghn_ps[:], trz[:, 0:128])
                nc.vector.tensor_add(tn[:], tn[:], gin_ps[:])
                nc.scalar.activation(out=tn[:], in_=tn[:], func=FX.Tanh)
                # h1 = n + z * (h0 - n)
                nc.vector.tensor_sub(td[:], h0B_own[:], tn[:])
                nc.vector.tensor_mul(td[:], td[:], trz[:, 128:256])
                nc.vector.tensor_add(td[:], td[:], tn[:])

                # h1 slice back to T layout with the fp8 scale folded in:
                # h1T_ps = td.T @ (16*I)
                h1T_ps = gps.tile([128, B], f32, tag="h1T_ps")
                nc.tensor.matmul(h1T_ps[:], td[:], id16[:], start=True, stop=True)
                h1f8_own = singles.tile([128, B], f8, tag="h1f8_own")
                nc.vector.tensor_copy(h1f8_own[:], h1T_ps[:])

                # ---- AllGather #2: h1 slices (fp8) -> full h1T -----------
                cc2_in = dram.tile([128, B], f8, tag="cc2_in")
                cc2_out = dram.tile([KC * 128, B], f8, tag="cc2_out")
                nc.scalar.dma_start(out=cc2_in[:], in_=h1f8_own[:])
                nc.gpsimd.collective_compute(
                    "AllGather",
                    mybir.AluOpType.bypass,
                    replica_groups=[list(range(NC))],
                    ins=[cc2_in.opt()],
                    outs=[cc2_out.opt()],
                )
                co2 = cc2_out[:]
                nc.scalar.dma_start(
                    out=h1f8,
                    in_=bass.AP(
                        tensor=co2.tensor, offset=co2.offset,
                        ap=[[B, 128], [128 * B, KC], [1, B]],
                    ),
                )

            # ---- projection (fp8 DoubleRow) + exp-sum --------------------
            with tc.tile_pool(name="proj_ps", bufs=2, space="PSUM") as pps:
                for gidx, (g0, g1) in enumerate(GROUPS):
                    gw = g1 - g0
                    lg = pps.tile([B, 2048], f32, tag="lg", name="lg")
                    for kk in range(KK):
                        for so in range(0, gw, 512):
                            sw = 512
                            col = g0 + so
                            nc.tensor.matmul(
                                lg[:, so : so + sw],
                                h1f8[:, 2 * kk : 2 * kk + 2, :],
                                pw_sb[kk][:, :, col : col + sw],
                                start=(kk == 0), stop=False,
                                perf_mode=DR,
                            )
                    for so in range(0, gw, 512):
                        sw = 512
                        col = g0 + so
                        nc.tensor.matmul(
                            lg[:, so : so + sw],
                            ones_sb[:],
                            pb_sb[0:1, col : col + sw],
                            start=False, stop=True,
                        )
                    nc.vector.tensor_copy(logits_sb[:, g0:g1], lg[:, :gw])
                    nc.sync.dma_start(
                        out=logits[:, g0:g1], in_=logits_sb[:, g0:g1]
                    )
                    nc.scalar.activation(
                        out=scratch[:, :gw],
                        in_=lg[:, :gw],
                        func=FX.Exp,
                        scale=1.0 / LG_S,
                        accum_out=cs_t[:, gidx : gidx + 1],
                    )
                    if gidx == 1:
                        nc.vector.tensor_add(s_run[:], cs_t[:, 0:1], cs_t[:, 1:2])
                    elif gidx == 2:
                        nc.vector.tensor_add(s_run[:], s_run[:], cs_t[:, 2:3])

            nc.vector.tensor_add(s_run[:], s_run[:], cs_t[:, 3:4])
            nc.sync.dma_start(out=svec, in_=s_run[:])

    nc.compile()
    return nc


def kernel(input, hidden, emb, bridge_w, bridge_b, w_ih, w_hh, b_ih, b_hh,
           proj_w, proj_b):
    global _NC_CACHE, LAST_RESULT
    if _NC_CACHE is None:
        _NC_CACHE = _build()
    nc = _NC_CACHE

    input = np.asarray(input)
    hidden = np.asarray(hidden, dtype=np.float32)
    emb = np.asarray(emb, dtype=np.float32)
    bridge_w = np.asarray(bridge_w, dtype=np.float32)
    bridge_b = np.asarray(bridge_b, dtype=np.float32)
    w_ih = np.asarray(w_ih, dtype=np.float32)
    w_hh = np.asarray(w_hh, dtype=np.float32)
    b_ih = np.asarray(b_ih, dtype=np.float32)
    b_hh = np.asarray(b_hh, dtype=np.float32)
    proj_w = np.asarray(proj_w, dtype=np.float32)
    proj_b = np.asarray(proj_b, dtype=np.float32)

    x0 = np.maximum(emb[input[:, 0].astype(np.int64)], 0.0)   # [B, H] relu
    x0T_in = np.ascontiguousarray(
        x0.T.reshape(KC, 128, B).transpose(1, 0, 2).astype(BF))
    hidT = hidden.transpose(1, 0, 2)                          # [L, B, H]
    bw_in = np.ascontiguousarray(bridge_w.reshape(L, 1).astype(BF))
    bb_in = bridge_b.reshape(1, 1)
    ones_in = np.ones((1, B), dtype=BF)
    bsum = b_ih + b_hh

    in_maps = []
    for c in range(NC):
        cs = slice(c * 128, (c + 1) * 128)
        rows = np.concatenate([g * H + np.arange(c * 128, (c + 1) * 128)
                               for g in range(3)])
        lo, hi = c * VC, min((c + 1) * VC, V)
        pw_blk = proj_w[lo:hi]
        pb_blk = proj_b[lo:hi]
        if hi - lo < VC:
            pad = VC - (hi - lo)
            pw_blk = np.concatenate(
                [pw_blk, np.zeros((pad, H), np.float32)], axis=0)
            pb_blk = np.concatenate([pb_blk, np.full((pad,), NEG, np.float32)])
        brow_c = np.concatenate([
            bsum[rows[:256]], b_ih[rows[256:]], b_hh[rows[256:]],
        ]).reshape(1, 512)
        in_maps.append({
            "idbf": np.eye(128, dtype=BF),
            "id16s": np.ascontiguousarray((np.eye(B) * H1_S).astype(np.float32)),
            "hid": np.ascontiguousarray(hidT[:, :, cs].astype(BF)),
            "bw": bw_in,
            "bb": bb_in,
            "x0T": x0T_in,
            "wih": np.ascontiguousarray(
                w_ih[rows].T.reshape(KC, 128, G3).transpose(1, 0, 2).astype(BF)),
            "whh": np.ascontiguousarray(
                w_hh[rows].T.reshape(KC, 128, G3).transpose(1, 0, 2).astype(BF)),
            "brow": np.ascontiguousarray(brow_c.astype(np.float32)),
            "ones1": ones_in,
            "pwT": np.ascontiguousarray(
                (pw_blk.T * PW_S).reshape(KK, 2, 128, VC)
                .transpose(0, 2, 1, 3).astype(F8)),
            "pb": np.ascontiguousarray(
                (pb_blk * LG_S).reshape(1, VC).astype(BF)),
        })

    res = run_bass_kernel_spmd(nc, in_maps, list(range(NC)))
    LAST_RESULT = res

    logits_full = np.concatenate(
        [res.results[c]["logits"].astype(np.float32) for c in range(NC)], axis=1
    )[:, :V] * (1.0 / LG_S)
    s_all = np.stack([res.results[c]["svec"][:, 0].astype(np.float64)
                      for c in range(NC)])            # [NC, B]
    lse = np.log(s_all.sum(axis=0)).astype(np.float32)  # [B]
    logp = np.ascontiguousarray(logits_full - lse[:, None])
    return np.broadcast_to(logp[:, None, :], (B, L - 1, V))
